# revision 21
# baseline (speedup 1.0000x reference)
"""Single-head causal attention (B=4, T=4096, C=768, H=64) on 8 trn2 NeuronCores.

Sharding: 2 cores per batch element, split over queries with a balanced
causal partition. Parity p=0 handles query rows [0:1024)+[3072:4096),
p=1 handles [1024:3072) — equal causal work (72 key-block iterations each).
Each core receives x[b] pre-transposed to [C, T] bf16, computes K/V for the
full sequence and Q for its own rows on-device, then runs blockwise
softmax(Q K^T / sqrt(C)) V with a ones-column appended to V so the softmax
denominator falls out of the same matmul (scores are O(1), so no running
max is needed).

Because the two parities need different compile-time loop structures, we
build two Bass programs and dispatch them as two concurrent 4-core PJRT
launches on disjoint device subsets.
"""

import numpy as np
import ml_dtypes

B, T, C, H = 4, 4096, 768, 64
N_CORES = 8
QCHUNK = 512
KB = 128
NC_CHUNKS = C // 128
SCALE = 1.0 / float(np.sqrt(np.float32(C)))

OWN_Q0 = {0: [0, 512, 3072, 3584], 1: [1024, 1536, 2048, 2560]}

_BF16 = ml_dtypes.bfloat16
_CACHE = {}


# ---------------------------------------------------------------------------
# walrus in this toolchain rejects >1 sem-wait on CTRL-class instructions;
# split the TileContext exit-drain waits across a chain of sync NOPs.
def _apply_tile_patch():
    import concourse.tile as tile_mod
    from concourse import mybir

    if getattr(tile_mod.TileContext, "_drain_patched", False):
        return

    def _patched(self, tick_clock, wait_clock):
        nc = self.nc
        probe = nc.sync.nop(nofuse=True)
        wait_clock.add_sem_waits(
            probe.ins, tile_mod.ScopedClock({None: tick_clock.global_clock})
        )
        si = probe.ins.sync_info
        waits = list(si.on_wait) if si and si.on_wait else []
        if len(waits) > 1:
            si.on_wait[:] = waits[:1]
            for w in waits[1:]:
                nop = nc.sync.nop(nofuse=True)
                if nop.ins.sync_info is None:
                    nop.ins.sync_info = mybir.SyncInfo(on_wait=[w], on_update=[])
                else:
                    nop.ins.sync_info.on_wait[:] = [w]
        nc.sync.drain()
        nc.all_engine_barrier()
        assert self.sems is not None
        popped = nc._tile_sem_poison_stack.pop()
        assert popped is self._sem_poison
        nc.clear_and_free_semaphores(list(self.sems.allocated().values()))
        nc.all_engine_barrier()

    tile_mod.TileContext._drain_and_barrier = _patched
    tile_mod.TileContext._drain_patched = True


def _n_kb(q0):
    return (q0 + QCHUNK) // KB


_MSW_CTR = [0]


def _split_multiwaits(nc):
    """walrus here allows only one sem-wait per instruction: move excess
    waits onto same-engine NOPs inserted immediately before."""
    from concourse import mybir

    for f in nc.m.functions:
        for bb in f.blocks:
            new_insts = []
            for inst in bb.instructions:
                si = inst.sync_info
                if si and si.on_wait and len(si.on_wait) > 1:
                    waits = list(si.on_wait)
                    for w in waits[:-1]:
                        _MSW_CTR[0] += 1
                        nop = mybir.InstNoOp(
                            name=f"I-msw{_MSW_CTR[0]}",
                            engine=inst.engine,
                            bass_nofuse=True,
                            sync_info=mybir.SyncInfo(on_wait=[w], on_update=[]),
                        )
                        new_insts.append(nop)
                    si.on_wait[:] = [waits[-1]]
                new_insts.append(inst)
            bb.instructions[:] = new_insts


def _build(parity):
    import concourse.bass as bass
    import concourse.tile as tile
    from concourse import mybir

    _apply_tile_patch()

    bf16 = mybir.dt.bfloat16
    f32 = mybir.dt.float32

    nc = bass.Bass()
    xT_d = nc.dram_tensor("xT", [C, T], bf16, kind="ExternalInput")
    wkv_d = nc.dram_tensor("wkv", [C, 128], bf16, kind="ExternalInput")
    wq_d = nc.dram_tensor("wq", [C, H], bf16, kind="ExternalInput")
    mask_d = nc.dram_tensor("mask", [128, 128], bf16, kind="ExternalInput")
    idb_d = nc.dram_tensor("idb", [128, 128], bf16, kind="ExternalInput")
    idf_d = nc.dram_tensor("idf", [128, 128], f32, kind="ExternalInput")
    y_d = nc.dram_tensor("y", [4 * QCHUNK, H], f32, kind="ExternalOutput")

    q0s = OWN_Q0[parity]
    n_tblk = T // KB

    with tile.TileContext(nc) as tc:
        with (
            tc.tile_pool(name="big", bufs=1) as big,
            tc.tile_pool(name="small", bufs=1) as small,
            tc.tile_pool(name="p_sb", bufs=4) as p_pool,
            tc.tile_pool(name="o_sb", bufs=2) as o_pool,
            tc.tile_pool(name="outp", bufs=4) as out_pool,
            tc.tile_pool(name="ps_kv", bufs=2, space="PSUM") as ps_kv,
            tc.tile_pool(name="ps_s", bufs=2, space="PSUM") as ps_s,
            tc.tile_pool(name="ps_o", bufs=1, space="PSUM") as ps_o,
            tc.tile_pool(name="ps_tr", bufs=1, space="PSUM") as ps_tr,
        ):
            # ---- static inputs -------------------------------------------
            # This parity only ever attends to the first n_kb_max key blocks;
            # x columns beyond that are only needed if it owns queries there.
            n_kb_max = _n_kb(q0s[-1])
            n_t_max = n_kb_max * KB // QCHUNK   # kT/vT 512-chunks to project
            cols_needed = max(n_kb_max * KB, q0s[-1] + QCHUNK)
            # xT loaded T-piece-major so the first projection chunk's inputs
            # arrive after ~1.5 MiB instead of the full 6 MiB.
            # small tensors first: every matmul depends on the weights, so
            # they must not queue behind 6 MiB of x on the serial DMA queue.
            wkv = small.tile([128, NC_CHUNKS, 128], bf16)
            nc.sync.dma_start(out=wkv, in_=wkv_d.rearrange("(n p) m -> p n m", p=128))
            wq = small.tile([128, NC_CHUNKS, H], bf16)
            nc.sync.dma_start(out=wq, in_=wq_d.rearrange("(n p) m -> p n m", p=128))
            mask = small.tile([128, 128], bf16)
            nc.gpsimd.dma_start(out=mask, in_=mask_d[:, :])
            idb = small.tile([128, 128], bf16)
            nc.gpsimd.dma_start(out=idb, in_=idb_d[:, :])
            idf = small.tile([128, 128], f32)
            nc.gpsimd.dma_start(out=idf, in_=idf_d[:, :])

            xT = big.tile([128, NC_CHUNKS, T], bf16)
            TP = 1024
            for tp in range((cols_needed + TP - 1) // TP):
                for c in range(NC_CHUNKS):
                    eng = nc.sync if c % 2 == 0 else nc.gpsimd
                    eng.dma_start(
                        out=xT[:, c, tp * TP:(tp + 1) * TP],
                        in_=xT_d[c * 128:(c + 1) * 128, tp * TP:(tp + 1) * TP])

            kT = big.tile([128, T], bf16)
            vT = big.tile([64, T], bf16)
            qT = big.tile([128, 4 * QCHUNK], bf16)
            vaug = big.tile([128, n_tblk, H + 1], bf16)
            nc.vector.memset(vaug[:, :, H:H + 1], 1.0)

            # ---- emission helpers (software-pipelined order so each
            # attention chunk's PE work follows only the projections it
            # actually needs — PE executes its stream in program order) ----
            def emit_kv_proj(t):
                sl = slice(t * QCHUNK, (t + 1) * QCHUNK)
                pkv = ps_kv.tile([128, QCHUNK], f32, tag="kv")
                for c in range(NC_CHUNKS):
                    nc.tensor.matmul(pkv, wkv[:, c, :], xT[:, c, sl],
                                     start=(c == 0), stop=(c == NC_CHUNKS - 1))
                nc.vector.tensor_copy(out=kT[0:64, sl], in_=pkv[0:64, :])
                nc.vector.tensor_copy(out=vT[:, sl], in_=pkv[64:128, :])
                nc.gpsimd.dma_start(out=kT[64:128, sl], in_=kT[0:64, sl])

            def emit_q_proj(i, q0):
                sl = slice(q0, q0 + QCHUNK)
                osl = slice(i * QCHUNK, (i + 1) * QCHUNK)
                pq = ps_kv.tile([64, QCHUNK], f32, tag="kv")
                for c in range(NC_CHUNKS):
                    nc.tensor.matmul(pq, wq[:, c, :], xT[:, c, sl],
                                     start=(c == 0), stop=(c == NC_CHUNKS - 1))
                nc.vector.tensor_copy(out=qT[0:64, osl], in_=pq)
                if _n_kb(q0) >= 12:
                    nc.gpsimd.dma_start(out=qT[64:128, osl], in_=qT[0:64, osl])

            def emit_v_block(kb):
                ptr = ps_tr.tile([128, 64], bf16, tag="tr")
                nc.tensor.transpose(ptr, vT[:, kb * 128:(kb + 1) * 128],
                                    idb[0:64, 0:64])
                nc.vector.tensor_copy(out=vaug[:, kb, 0:H], in_=ptr)

            def emit_attention_pairs(i, q0, o_ps, kb_lo, kb_hi):
                qbase = i * QCHUNK
                nkb = _n_kb(q0)
                d0 = q0 // KB
                for pi in range(kb_lo // 2, kb_hi // 2):
                    kb1, kb2 = 2 * pi, 2 * pi + 1
                    offs, ns = [], []
                    for kb in (kb1, kb2):
                        d = kb - d0
                        off = 0 if d < 0 else d * KB
                        offs.append(off)
                        ns.append(QCHUNK - off)
                    s_ps = ps_s.tile([128, 2 * QCHUNK], f32)
                    packed = nkb >= 12
                    for half, kb in enumerate((kb1, kb2)):
                        rg = half * 64 if packed else 0
                        nc.tensor.matmul(
                            s_ps[:, half * QCHUNK: half * QCHUNK + ns[half]],
                            kT[rg:rg + 64, kb * KB:(kb + 1) * KB],
                            qT[rg:rg + 64, qbase + offs[half]: qbase + QCHUNK],
                            tile_position=(rg, 0))
                    p_sb = p_pool.tile([128, 2 * QCHUNK], bf16)
                    fd = QCHUNK + ns[1]
                    nc.scalar.activation(out=p_sb[:, 0:fd], in_=s_ps[:, 0:fd],
                                         func=mybir.ActivationFunctionType.Exp,
                                         scale=float(SCALE))
                    for half, kb in enumerate((kb1, kb2)):
                        if kb >= d0:
                            base = half * QCHUNK
                            nc.vector.tensor_mul(p_sb[:, base:base + KB],
                                                 p_sb[:, base:base + KB], mask)
                        nc.tensor.matmul(
                            o_ps[:, offs[half]:QCHUNK],
                            vaug[:, kb, :],
                            p_sb[:, half * QCHUNK: half * QCHUNK + ns[half]],
                            start=(kb == 0), stop=(kb == nkb - 1),
                            skip_group_check=True)

            def emit_epilogue(i, q0, o_ps):
                qbase = i * QCHUNK
                o_sb = o_pool.tile([H + 1, QCHUNK], f32)
                nc.vector.tensor_copy(out=o_sb, in_=o_ps)
                out_sb = out_pool.tile([128, QCHUNK // 128, H], f32)
                for j in range(QCHUNK // 128):
                    tr = ps_tr.tile([128, H + 1], f32, tag="tr")
                    nc.tensor.transpose(tr, o_sb[:, j * 128:(j + 1) * 128],
                                        idf[0:H + 1, 0:H + 1])
                    rinv = out_pool.tile([128, 1], f32)
                    nc.vector.reciprocal(rinv, tr[:, H:H + 1])
                    nc.vector.tensor_scalar_mul(out_sb[:, j, :], tr[:, 0:H], rinv)
                nc.sync.dma_start(
                    out=y_d[qbase:qbase + QCHUNK, :].rearrange(
                        "(j p) h -> p j h", p=128),
                    in_=out_sb)

            # ---- pipelined emission: attention pairs interleave with the
            # projections at t-chunk granularity so ScalarE always has exp
            # work while TensorE projects later K/V chunks. q-proj is
            # deferred past the first kv-proj so chunks whose query columns
            # ride the last DMA pieces don't stall the PE stream early. ----
            emitted_t = 0
            emitted_tr = 0
            for ci, q0 in enumerate(q0s):
                nkb = _n_kb(q0)
                o_ps = ps_o.tile([H + 1, QCHUNK], f32)
                done_kb = 0
                q_emitted = False
                while done_kb < nkb:
                    if emitted_t * 4 < nkb:
                        emit_kv_proj(emitted_t)
                        emitted_t += 1
                    hi = min(4 * emitted_t, nkb)
                    while emitted_tr < hi:
                        emit_v_block(emitted_tr)
                        emitted_tr += 1
                    if not q_emitted:
                        emit_q_proj(ci, q0)
                        q_emitted = True
                    emit_attention_pairs(ci, q0, o_ps, done_kb, hi)
                    done_kb = hi
                emit_epilogue(ci, q0, o_ps)
    _split_multiwaits(nc)
    return nc


# ---------------------------------------------------------------------------
# PJRT launcher for one Bass program on an arbitrary device subset.
def _make_launcher(nc, devices):
    import jax
    from jax.sharding import Mesh, PartitionSpec
    from jax.experimental.shard_map import shard_map
    import concourse.mybir as mybir
    from concourse.bass2jax import (
        install_neuronx_cc_hook, _bass_exec_p, partition_id_tensor)

    install_neuronx_cc_hook()

    partition_name = nc.partition_id_tensor.name if nc.partition_id_tensor else None
    in_names, out_names, out_avals, zero_outs = [], [], [], []
    for alloc in nc.m.functions[0].allocations:
        if not isinstance(alloc, mybir.MemoryLocationSet):
            continue
        name = alloc.memorylocations[0].name
        if alloc.kind == "ExternalInput":
            if name != partition_name:
                in_names.append(name)
        elif alloc.kind == "ExternalOutput":
            out_names.append(name)
            shape = tuple(alloc.tensor_shape)
            dtype = mybir.dt.np(alloc.dtype)
            out_avals.append(jax.core.ShapedArray(shape, dtype))
            zero_outs.append(np.zeros(shape, dtype))
    n_params = len(in_names)
    n_outs = len(out_avals)
    all_names = in_names + out_names
    if partition_name is not None:
        all_names = all_names + [partition_name]
    donate = tuple(range(n_params, n_params + n_outs))

    def _body(*args):
        operands = list(args)
        if partition_name is not None:
            operands.append(partition_id_tensor())
        outs = _bass_exec_p.bind(
            *operands,
            out_avals=tuple(out_avals),
            in_names=tuple(all_names),
            out_names=tuple(out_names),
            lowering_input_output_aliases=(),
            sim_require_finite=True,
            sim_require_nnan=True,
            nc=nc,
        )
        return tuple(outs)

    n_dev = len(devices)
    mesh = Mesh(np.asarray(devices), ("core",))
    in_specs = (PartitionSpec("core"),) * (n_params + n_outs)
    out_specs = (PartitionSpec("core"),) * n_outs
    fn = jax.jit(
        shard_map(_body, mesh=mesh, in_specs=in_specs, out_specs=out_specs,
                  check_rep=False),
        donate_argnums=donate, keep_unused=True)

    def run(in_maps):
        assert len(in_maps) == n_dev
        concat_in = [
            np.concatenate([np.asarray(in_maps[c][nm]) for c in range(n_dev)], axis=0)
            for nm in in_names
        ]
        concat_zero = [
            np.concatenate([z] * n_dev, axis=0) for z in zero_outs
        ]
        outs = fn(*concat_in, *concat_zero)
        return outs, out_names

    return run


def _get_launchers():
    if "launchers" not in _CACHE:
        import jax
        devs = jax.devices()
        nc0 = _build(0)
        nc1 = _build(1)
        # parity-0 program on devices [0,2,4,6] (batches 0-3),
        # parity-1 on [1,3,5,7].
        run0 = _make_launcher(nc0, [devs[i] for i in (0, 2, 4, 6)])
        run1 = _make_launcher(nc1, [devs[i] for i in (1, 3, 5, 7)])
        _CACHE["launchers"] = (run0, run1)
        _CACHE["ncs"] = (nc0, nc1)
    return _CACHE["launchers"]


def _prep_core_inputs(x, Wq, Wk, Wv):
    x = np.asarray(x, dtype=np.float32)
    wkv = np.concatenate([np.asarray(Wk, np.float32),
                          np.asarray(Wv, np.float32)], axis=1).astype(_BF16)
    wq = np.asarray(Wq, np.float32).astype(_BF16)
    mask = np.triu(np.ones((128, 128), np.float32)).astype(_BF16)
    idb = np.eye(128, dtype=np.float32).astype(_BF16)
    idf = np.eye(128, dtype=np.float32)
    per_batch_xT = [np.ascontiguousarray(x[b].T).astype(_BF16) for b in range(B)]
    common = {"wkv": wkv, "wq": wq, "mask": mask, "idb": idb, "idf": idf}
    maps0 = [{"xT": per_batch_xT[b], **common} for b in range(B)]
    maps1 = [{"xT": per_batch_xT[b], **common} for b in range(B)]
    return maps0, maps1


def kernel(x, Wq, Wk, Wv):
    run0, run1 = _get_launchers()
    maps0, maps1 = _prep_core_inputs(x, Wq, Wk, Wv)
    outs0, names0 = run0(maps0)          # async dispatch
    outs1, names1 = run1(maps1)
    y0 = np.asarray(outs0[names0.index("y")])   # blocks
    y1 = np.asarray(outs1[names1.index("y")])

    out = np.empty((B, T, H), dtype=np.float32)
    rows = 4 * QCHUNK
    for b in range(B):
        yb0 = y0[b * rows:(b + 1) * rows]
        yb1 = y1[b * rows:(b + 1) * rows]
        for i, q0 in enumerate(OWN_Q0[0]):
            out[b, q0:q0 + QCHUNK] = yb0[i * QCHUNK:(i + 1) * QCHUNK]
        for i, q0 in enumerate(OWN_Q0[1]):
            out[b, q0:q0 + QCHUNK] = yb1[i * QCHUNK:(i + 1) * QCHUNK]
    return out


# revision 22
# speedup vs baseline: 1.0442x; 1.0442x over previous
"""Single-head causal attention (B=4, T=4096, C=768, H=64) on 8 trn2 NeuronCores.

Sharding: 2 cores per batch element, split over queries with a balanced
causal partition. Parity p=0 handles query rows [0:1024)+[3072:4096),
p=1 handles [1024:3072) — equal causal work (72 key-block iterations each).
Each core receives x[b] pre-transposed to [C, T] bf16, computes K/V for the
full sequence and Q for its own rows on-device, then runs blockwise
softmax(Q K^T / sqrt(C)) V with a ones-column appended to V so the softmax
denominator falls out of the same matmul (scores are O(1), so no running
max is needed).

Because the two parities need different compile-time loop structures, we
build two Bass programs and dispatch them as two concurrent 4-core PJRT
launches on disjoint device subsets.
"""

import numpy as np
import ml_dtypes

B, T, C, H = 4, 4096, 768, 64
N_CORES = 8
QCHUNK = 512
KB = 128
NC_CHUNKS = C // 128
SCALE = 1.0 / float(np.sqrt(np.float32(C)))

OWN_Q0 = {0: [0, 512, 3072, 3584], 1: [1024, 1536, 2048, 2560]}

_BF16 = ml_dtypes.bfloat16
_CACHE = {}


# ---------------------------------------------------------------------------
# walrus in this toolchain rejects >1 sem-wait on CTRL-class instructions;
# split the TileContext exit-drain waits across a chain of sync NOPs.
def _apply_tile_patch():
    import concourse.tile as tile_mod
    from concourse import mybir

    if getattr(tile_mod.TileContext, "_drain_patched", False):
        return

    def _patched(self, tick_clock, wait_clock):
        nc = self.nc
        probe = nc.sync.nop(nofuse=True)
        wait_clock.add_sem_waits(
            probe.ins, tile_mod.ScopedClock({None: tick_clock.global_clock})
        )
        si = probe.ins.sync_info
        waits = list(si.on_wait) if si and si.on_wait else []
        if len(waits) > 1:
            si.on_wait[:] = waits[:1]
            for w in waits[1:]:
                nop = nc.sync.nop(nofuse=True)
                if nop.ins.sync_info is None:
                    nop.ins.sync_info = mybir.SyncInfo(on_wait=[w], on_update=[])
                else:
                    nop.ins.sync_info.on_wait[:] = [w]
        nc.sync.drain()
        nc.all_engine_barrier()
        assert self.sems is not None
        popped = nc._tile_sem_poison_stack.pop()
        assert popped is self._sem_poison
        nc.clear_and_free_semaphores(list(self.sems.allocated().values()))
        nc.all_engine_barrier()

    tile_mod.TileContext._drain_and_barrier = _patched
    tile_mod.TileContext._drain_patched = True


def _n_kb(q0):
    return (q0 + QCHUNK) // KB


_MSW_CTR = [0]


def _split_multiwaits(nc):
    """walrus here allows only one sem-wait per instruction: move excess
    waits onto same-engine NOPs inserted immediately before."""
    from concourse import mybir

    for f in nc.m.functions:
        for bb in f.blocks:
            new_insts = []
            for inst in bb.instructions:
                si = inst.sync_info
                if si and si.on_wait and len(si.on_wait) > 1:
                    waits = list(si.on_wait)
                    for w in waits[:-1]:
                        _MSW_CTR[0] += 1
                        nop = mybir.InstNoOp(
                            name=f"I-msw{_MSW_CTR[0]}",
                            engine=inst.engine,
                            bass_nofuse=True,
                            sync_info=mybir.SyncInfo(on_wait=[w], on_update=[]),
                        )
                        new_insts.append(nop)
                    si.on_wait[:] = [waits[-1]]
                new_insts.append(inst)
            bb.instructions[:] = new_insts


def _build(parity):
    import concourse.bass as bass
    import concourse.tile as tile
    from concourse import mybir

    _apply_tile_patch()

    bf16 = mybir.dt.bfloat16
    f32 = mybir.dt.float32

    nc = bass.Bass()
    xT_d = nc.dram_tensor("xT", [C, T], bf16, kind="ExternalInput")
    wkv_d = nc.dram_tensor("wkv", [C, 128], bf16, kind="ExternalInput")
    wq_d = nc.dram_tensor("wq", [C, H], bf16, kind="ExternalInput")
    mask_d = nc.dram_tensor("mask", [128, 128], bf16, kind="ExternalInput")
    idb_d = nc.dram_tensor("idb", [128, 128], bf16, kind="ExternalInput")
    idf_d = nc.dram_tensor("idf", [128, 128], f32, kind="ExternalInput")
    y_d = nc.dram_tensor("y", [4 * QCHUNK, H], f32, kind="ExternalOutput")

    q0s = OWN_Q0[parity]
    n_tblk = T // KB

    with tile.TileContext(nc) as tc:
        with (
            tc.tile_pool(name="big", bufs=1) as big,
            tc.tile_pool(name="small", bufs=1) as small,
            tc.tile_pool(name="p_sb", bufs=4) as p_pool,
            tc.tile_pool(name="o_sb", bufs=2) as o_pool,
            tc.tile_pool(name="outp", bufs=4) as out_pool,
            tc.tile_pool(name="ps_kv", bufs=2, space="PSUM") as ps_kv,
            tc.tile_pool(name="ps_s", bufs=2, space="PSUM") as ps_s,
            tc.tile_pool(name="ps_o", bufs=1, space="PSUM") as ps_o,
            tc.tile_pool(name="ps_tr", bufs=1, space="PSUM") as ps_tr,
        ):
            # ---- static inputs -------------------------------------------
            # This parity only ever attends to the first n_kb_max key blocks;
            # x columns beyond that are only needed if it owns queries there.
            n_kb_max = _n_kb(q0s[-1])
            n_t_max = n_kb_max * KB // QCHUNK   # kT/vT 512-chunks to project
            cols_needed = max(n_kb_max * KB, q0s[-1] + QCHUNK)
            # xT loaded T-piece-major so the first projection chunk's inputs
            # arrive after ~1.5 MiB instead of the full 6 MiB.
            # small tensors first: every matmul depends on the weights, so
            # they must not queue behind 6 MiB of x on the serial DMA queue.
            wkv = small.tile([128, NC_CHUNKS, 128], bf16)
            nc.sync.dma_start(out=wkv, in_=wkv_d.rearrange("(n p) m -> p n m", p=128))
            wq = small.tile([128, NC_CHUNKS, H], bf16)
            nc.sync.dma_start(out=wq, in_=wq_d.rearrange("(n p) m -> p n m", p=128))
            mask = small.tile([128, 128], bf16)
            nc.gpsimd.dma_start(out=mask, in_=mask_d[:, :])
            idb = small.tile([128, 128], bf16)
            nc.gpsimd.dma_start(out=idb, in_=idb_d[:, :])
            idf = small.tile([128, 128], f32)
            nc.gpsimd.dma_start(out=idf, in_=idf_d[:, :])

            xT = big.tile([128, NC_CHUNKS, T], bf16)
            TP = 1024
            for tp in range((cols_needed + TP - 1) // TP):
                for c in range(NC_CHUNKS):
                    eng = nc.sync if c % 2 == 0 else nc.gpsimd
                    eng.dma_start(
                        out=xT[:, c, tp * TP:(tp + 1) * TP],
                        in_=xT_d[c * 128:(c + 1) * 128, tp * TP:(tp + 1) * TP])

            kT = big.tile([128, T], bf16)
            vT = big.tile([64, T], bf16)
            qT = big.tile([128, 4 * QCHUNK], bf16)
            vaug = big.tile([128, n_tblk, H + 1], bf16)
            nc.vector.memset(vaug[:, :, H:H + 1], 1.0)

            # ---- emission helpers (software-pipelined order so each
            # attention chunk's PE work follows only the projections it
            # actually needs — PE executes its stream in program order) ----
            def emit_kv_proj(t):
                sl = slice(t * QCHUNK, (t + 1) * QCHUNK)
                pkv = ps_kv.tile([128, QCHUNK], f32, tag="kv")
                for c in range(NC_CHUNKS):
                    nc.tensor.matmul(pkv, wkv[:, c, :], xT[:, c, sl],
                                     start=(c == 0), stop=(c == NC_CHUNKS - 1))
                nc.vector.tensor_copy(out=kT[0:64, sl], in_=pkv[0:64, :])
                nc.vector.tensor_copy(out=vT[:, sl], in_=pkv[64:128, :])
                nc.gpsimd.dma_start(out=kT[64:128, sl], in_=kT[0:64, sl])

            def emit_q_proj(i, q0):
                sl = slice(q0, q0 + QCHUNK)
                osl = slice(i * QCHUNK, (i + 1) * QCHUNK)
                pq = ps_kv.tile([64, QCHUNK], f32, tag="kv")
                for c in range(NC_CHUNKS):
                    nc.tensor.matmul(pq, wq[:, c, :], xT[:, c, sl],
                                     start=(c == 0), stop=(c == NC_CHUNKS - 1))
                nc.vector.tensor_copy(out=qT[0:64, osl], in_=pq)
                if _n_kb(q0) >= 12:
                    nc.gpsimd.dma_start(out=qT[64:128, osl], in_=qT[0:64, osl])

            def emit_v_block(kb):
                ptr = ps_tr.tile([128, 64], bf16, tag="tr")
                nc.tensor.transpose(ptr, vT[:, kb * 128:(kb + 1) * 128],
                                    idb[0:64, 0:64])
                nc.vector.tensor_copy(out=vaug[:, kb, 0:H], in_=ptr)

            def emit_attention(i, q0):
                qbase = i * QCHUNK
                nkb = _n_kb(q0)
                d0 = q0 // KB
                o_ps = ps_o.tile([H + 1, QCHUNK], f32)
                for pi in range(nkb // 2):
                    kb1, kb2 = 2 * pi, 2 * pi + 1
                    offs, ns = [], []
                    for kb in (kb1, kb2):
                        d = kb - d0
                        off = 0 if d < 0 else d * KB
                        offs.append(off)
                        ns.append(QCHUNK - off)
                    s_ps = ps_s.tile([128, 2 * QCHUNK], f32)
                    packed = nkb >= 12
                    for half, kb in enumerate((kb1, kb2)):
                        rg = half * 64 if packed else 0
                        nc.tensor.matmul(
                            s_ps[:, half * QCHUNK: half * QCHUNK + ns[half]],
                            kT[rg:rg + 64, kb * KB:(kb + 1) * KB],
                            qT[rg:rg + 64, qbase + offs[half]: qbase + QCHUNK],
                            tile_position=(rg, 0))
                    p_sb = p_pool.tile([128, 2 * QCHUNK], bf16)
                    fd = QCHUNK + ns[1]
                    nc.scalar.activation(out=p_sb[:, 0:fd], in_=s_ps[:, 0:fd],
                                         func=mybir.ActivationFunctionType.Exp,
                                         scale=float(SCALE))
                    for half, kb in enumerate((kb1, kb2)):
                        if kb >= d0:
                            base = half * QCHUNK
                            nc.vector.tensor_mul(p_sb[:, base:base + KB],
                                                 p_sb[:, base:base + KB], mask)
                        nc.tensor.matmul(
                            o_ps[:, offs[half]:QCHUNK],
                            vaug[:, kb, :],
                            p_sb[:, half * QCHUNK: half * QCHUNK + ns[half]],
                            start=(kb == 0), stop=(kb == nkb - 1),
                            skip_group_check=True)

                o_sb = o_pool.tile([H + 1, QCHUNK], f32)
                nc.vector.tensor_copy(out=o_sb, in_=o_ps)
                out_sb = out_pool.tile([128, QCHUNK // 128, H], f32)
                for j in range(QCHUNK // 128):
                    tr = ps_tr.tile([128, H + 1], f32, tag="tr")
                    nc.tensor.transpose(tr, o_sb[:, j * 128:(j + 1) * 128],
                                        idf[0:H + 1, 0:H + 1])
                    rinv = out_pool.tile([128, 1], f32)
                    nc.vector.reciprocal(rinv, tr[:, H:H + 1])
                    nc.vector.tensor_scalar_mul(out_sb[:, j, :], tr[:, 0:H], rinv)
                nc.sync.dma_start(
                    out=y_d[qbase:qbase + QCHUNK, :].rearrange(
                        "(j p) h -> p j h", p=128),
                    in_=out_sb)

            # ---- pipelined emission: each chunk's attention right after
            # the projections/transposes it needs --------------------------
            emitted_t = 0
            emitted_tr = 0
            for ci, q0 in enumerate(q0s):
                need_t = _n_kb(q0) * KB // QCHUNK
                while emitted_t < need_t:
                    emit_kv_proj(emitted_t)
                    emitted_t += 1
                emit_q_proj(ci, q0)
                while emitted_tr < _n_kb(q0):
                    emit_v_block(emitted_tr)
                    emitted_tr += 1
                emit_attention(ci, q0)
    _split_multiwaits(nc)
    return nc


# ---------------------------------------------------------------------------
# PJRT launcher for one Bass program on an arbitrary device subset.
def _make_launcher(nc, devices):
    import jax
    from jax.sharding import Mesh, PartitionSpec
    from jax.experimental.shard_map import shard_map
    import concourse.mybir as mybir
    from concourse.bass2jax import (
        install_neuronx_cc_hook, _bass_exec_p, partition_id_tensor)

    install_neuronx_cc_hook()

    partition_name = nc.partition_id_tensor.name if nc.partition_id_tensor else None
    in_names, out_names, out_avals, zero_outs = [], [], [], []
    for alloc in nc.m.functions[0].allocations:
        if not isinstance(alloc, mybir.MemoryLocationSet):
            continue
        name = alloc.memorylocations[0].name
        if alloc.kind == "ExternalInput":
            if name != partition_name:
                in_names.append(name)
        elif alloc.kind == "ExternalOutput":
            out_names.append(name)
            shape = tuple(alloc.tensor_shape)
            dtype = mybir.dt.np(alloc.dtype)
            out_avals.append(jax.core.ShapedArray(shape, dtype))
            zero_outs.append(np.zeros(shape, dtype))
    n_params = len(in_names)
    n_outs = len(out_avals)
    all_names = in_names + out_names
    if partition_name is not None:
        all_names = all_names + [partition_name]
    donate = tuple(range(n_params, n_params + n_outs))

    def _body(*args):
        operands = list(args)
        if partition_name is not None:
            operands.append(partition_id_tensor())
        outs = _bass_exec_p.bind(
            *operands,
            out_avals=tuple(out_avals),
            in_names=tuple(all_names),
            out_names=tuple(out_names),
            lowering_input_output_aliases=(),
            sim_require_finite=True,
            sim_require_nnan=True,
            nc=nc,
        )
        return tuple(outs)

    n_dev = len(devices)
    mesh = Mesh(np.asarray(devices), ("core",))
    in_specs = (PartitionSpec("core"),) * (n_params + n_outs)
    out_specs = (PartitionSpec("core"),) * n_outs
    fn = jax.jit(
        shard_map(_body, mesh=mesh, in_specs=in_specs, out_specs=out_specs,
                  check_rep=False),
        donate_argnums=donate, keep_unused=True)

    def run(in_maps):
        assert len(in_maps) == n_dev
        concat_in = [
            np.concatenate([np.asarray(in_maps[c][nm]) for c in range(n_dev)], axis=0)
            for nm in in_names
        ]
        concat_zero = [
            np.concatenate([z] * n_dev, axis=0) for z in zero_outs
        ]
        outs = fn(*concat_in, *concat_zero)
        return outs, out_names

    return run


def _get_launchers():
    if "launchers" not in _CACHE:
        import jax
        devs = jax.devices()
        nc0 = _build(0)
        nc1 = _build(1)
        # parity-0 program on devices [0,2,4,6] (batches 0-3),
        # parity-1 on [1,3,5,7].
        run0 = _make_launcher(nc0, [devs[i] for i in (0, 2, 4, 6)])
        run1 = _make_launcher(nc1, [devs[i] for i in (1, 3, 5, 7)])
        _CACHE["launchers"] = (run0, run1)
        _CACHE["ncs"] = (nc0, nc1)
    return _CACHE["launchers"]


def _prep_core_inputs(x, Wq, Wk, Wv):
    x = np.asarray(x, dtype=np.float32)
    wkv = np.concatenate([np.asarray(Wk, np.float32),
                          np.asarray(Wv, np.float32)], axis=1).astype(_BF16)
    wq = np.asarray(Wq, np.float32).astype(_BF16)
    mask = np.triu(np.ones((128, 128), np.float32)).astype(_BF16)
    idb = np.eye(128, dtype=np.float32).astype(_BF16)
    idf = np.eye(128, dtype=np.float32)
    per_batch_xT = [np.ascontiguousarray(x[b].T).astype(_BF16) for b in range(B)]
    common = {"wkv": wkv, "wq": wq, "mask": mask, "idb": idb, "idf": idf}
    maps0 = [{"xT": per_batch_xT[b], **common} for b in range(B)]
    maps1 = [{"xT": per_batch_xT[b], **common} for b in range(B)]
    return maps0, maps1


def kernel(x, Wq, Wk, Wv):
    run0, run1 = _get_launchers()
    maps0, maps1 = _prep_core_inputs(x, Wq, Wk, Wv)
    outs0, names0 = run0(maps0)          # async dispatch
    outs1, names1 = run1(maps1)
    y0 = np.asarray(outs0[names0.index("y")])   # blocks
    y1 = np.asarray(outs1[names1.index("y")])

    out = np.empty((B, T, H), dtype=np.float32)
    rows = 4 * QCHUNK
    for b in range(B):
        yb0 = y0[b * rows:(b + 1) * rows]
        yb1 = y1[b * rows:(b + 1) * rows]
        for i, q0 in enumerate(OWN_Q0[0]):
            out[b, q0:q0 + QCHUNK] = yb0[i * QCHUNK:(i + 1) * QCHUNK]
        for i, q0 in enumerate(OWN_Q0[1]):
            out[b, q0:q0 + QCHUNK] = yb1[i * QCHUNK:(i + 1) * QCHUNK]
    return out


# revision 23
# speedup vs baseline: 1.0518x; 1.0072x over previous
"""Single-head causal attention (B=4, T=4096, C=768, H=64) on 8 trn2 NeuronCores.

Sharding: 2 cores per batch element, split over queries with a balanced
causal partition. Parity p=0 handles query rows [0:1024)+[3072:4096),
p=1 handles [1024:3072) — equal causal work (72 key-block iterations each).
Each core receives x[b] pre-transposed to [C, T] bf16, computes K/V for the
full sequence and Q for its own rows on-device, then runs blockwise
softmax(Q K^T / sqrt(C)) V with a ones-column appended to V so the softmax
denominator falls out of the same matmul (scores are O(1), so no running
max is needed).

Because the two parities need different compile-time loop structures, we
build two Bass programs and dispatch them as two concurrent 4-core PJRT
launches on disjoint device subsets.
"""

import numpy as np
import ml_dtypes

B, T, C, H = 4, 4096, 768, 64
N_CORES = 8
QCHUNK = 512
KB = 128
NC_CHUNKS = C // 128
SCALE = 1.0 / float(np.sqrt(np.float32(C)))

OWN_Q0 = {0: [0, 512, 3072, 3584], 1: [1024, 1536, 2048, 2560]}

_BF16 = ml_dtypes.bfloat16
_CACHE = {}


# ---------------------------------------------------------------------------
# walrus in this toolchain rejects >1 sem-wait on CTRL-class instructions;
# split the TileContext exit-drain waits across a chain of sync NOPs.
def _apply_tile_patch():
    import concourse.tile as tile_mod
    from concourse import mybir

    if getattr(tile_mod.TileContext, "_drain_patched", False):
        return

    def _patched(self, tick_clock, wait_clock):
        nc = self.nc
        probe = nc.sync.nop(nofuse=True)
        wait_clock.add_sem_waits(
            probe.ins, tile_mod.ScopedClock({None: tick_clock.global_clock})
        )
        si = probe.ins.sync_info
        waits = list(si.on_wait) if si and si.on_wait else []
        if len(waits) > 1:
            si.on_wait[:] = waits[:1]
            for w in waits[1:]:
                nop = nc.sync.nop(nofuse=True)
                if nop.ins.sync_info is None:
                    nop.ins.sync_info = mybir.SyncInfo(on_wait=[w], on_update=[])
                else:
                    nop.ins.sync_info.on_wait[:] = [w]
        nc.sync.drain()
        nc.all_engine_barrier()
        assert self.sems is not None
        popped = nc._tile_sem_poison_stack.pop()
        assert popped is self._sem_poison
        nc.clear_and_free_semaphores(list(self.sems.allocated().values()))
        nc.all_engine_barrier()

    tile_mod.TileContext._drain_and_barrier = _patched
    tile_mod.TileContext._drain_patched = True


def _n_kb(q0):
    return (q0 + QCHUNK) // KB


_MSW_CTR = [0]


def _split_multiwaits(nc):
    """walrus here allows only one sem-wait per instruction: move excess
    waits onto same-engine NOPs inserted immediately before."""
    from concourse import mybir

    for f in nc.m.functions:
        for bb in f.blocks:
            new_insts = []
            for inst in bb.instructions:
                si = inst.sync_info
                if si and si.on_wait and len(si.on_wait) > 1:
                    waits = list(si.on_wait)
                    for w in waits[:-1]:
                        _MSW_CTR[0] += 1
                        nop = mybir.InstNoOp(
                            name=f"I-msw{_MSW_CTR[0]}",
                            engine=inst.engine,
                            bass_nofuse=True,
                            sync_info=mybir.SyncInfo(on_wait=[w], on_update=[]),
                        )
                        new_insts.append(nop)
                    si.on_wait[:] = [waits[-1]]
                new_insts.append(inst)
            bb.instructions[:] = new_insts


def _build(parity):
    import concourse.bass as bass
    import concourse.tile as tile
    from concourse import mybir

    _apply_tile_patch()

    bf16 = mybir.dt.bfloat16
    f32 = mybir.dt.float32

    nc = bass.Bass()
    xT_d = nc.dram_tensor("xT", [C, T], bf16, kind="ExternalInput")
    wkv_d = nc.dram_tensor("wkv", [C, 128], bf16, kind="ExternalInput")
    wq_d = nc.dram_tensor("wq", [C, H], bf16, kind="ExternalInput")
    mask_d = nc.dram_tensor("mask", [128, 128], bf16, kind="ExternalInput")
    idb_d = nc.dram_tensor("idb", [128, 128], bf16, kind="ExternalInput")
    idf_d = nc.dram_tensor("idf", [128, 128], f32, kind="ExternalInput")
    y_d = nc.dram_tensor("y", [4 * QCHUNK, H], f32, kind="ExternalOutput")

    q0s = OWN_Q0[parity]
    n_tblk = T // KB

    with tile.TileContext(nc) as tc:
        with (
            tc.tile_pool(name="big", bufs=1) as big,
            tc.tile_pool(name="small", bufs=1) as small,
            tc.tile_pool(name="p_sb", bufs=6) as p_pool,
            tc.tile_pool(name="o_sb", bufs=3) as o_pool,
            tc.tile_pool(name="outp", bufs=6) as out_pool,
            tc.tile_pool(name="ps_kv", bufs=2, space="PSUM") as ps_kv,
            tc.tile_pool(name="ps_s", bufs=2, space="PSUM") as ps_s,
            tc.tile_pool(name="ps_o", bufs=1, space="PSUM") as ps_o,
            tc.tile_pool(name="ps_tr", bufs=1, space="PSUM") as ps_tr,
        ):
            # ---- static inputs -------------------------------------------
            # This parity only ever attends to the first n_kb_max key blocks;
            # x columns beyond that are only needed if it owns queries there.
            n_kb_max = _n_kb(q0s[-1])
            n_t_max = n_kb_max * KB // QCHUNK   # kT/vT 512-chunks to project
            cols_needed = max(n_kb_max * KB, q0s[-1] + QCHUNK)
            # xT loaded T-piece-major so the first projection chunk's inputs
            # arrive after ~1.5 MiB instead of the full 6 MiB.
            # small tensors first: every matmul depends on the weights, so
            # they must not queue behind 6 MiB of x on the serial DMA queue.
            wkv = small.tile([128, NC_CHUNKS, 128], bf16)
            nc.sync.dma_start(out=wkv, in_=wkv_d.rearrange("(n p) m -> p n m", p=128))
            wq = small.tile([128, NC_CHUNKS, H], bf16)
            nc.sync.dma_start(out=wq, in_=wq_d.rearrange("(n p) m -> p n m", p=128))
            mask = small.tile([128, 128], bf16)
            nc.gpsimd.dma_start(out=mask, in_=mask_d[:, :])
            idb = small.tile([128, 128], bf16)
            nc.gpsimd.dma_start(out=idb, in_=idb_d[:, :])
            idf = small.tile([128, 128], f32)
            nc.gpsimd.dma_start(out=idf, in_=idf_d[:, :])

            xT = big.tile([128, NC_CHUNKS, T], bf16)
            TP = 1024
            for tp in range((cols_needed + TP - 1) // TP):
                for c in range(NC_CHUNKS):
                    eng = nc.sync if c % 2 == 0 else nc.gpsimd
                    eng.dma_start(
                        out=xT[:, c, tp * TP:(tp + 1) * TP],
                        in_=xT_d[c * 128:(c + 1) * 128, tp * TP:(tp + 1) * TP])

            kT = big.tile([128, T], bf16)
            vT = big.tile([64, T], bf16)
            qT = big.tile([128, 4 * QCHUNK], bf16)
            vaug = big.tile([128, n_tblk, H + 1], bf16)
            nc.vector.memset(vaug[:, :, H:H + 1], 1.0)

            # ---- emission helpers (software-pipelined order so each
            # attention chunk's PE work follows only the projections it
            # actually needs — PE executes its stream in program order) ----
            def emit_kv_proj(t):
                sl = slice(t * QCHUNK, (t + 1) * QCHUNK)
                pkv = ps_kv.tile([128, QCHUNK], f32, tag="kv")
                for c in range(NC_CHUNKS):
                    nc.tensor.matmul(pkv, wkv[:, c, :], xT[:, c, sl],
                                     start=(c == 0), stop=(c == NC_CHUNKS - 1))
                nc.vector.tensor_copy(out=kT[0:64, sl], in_=pkv[0:64, :])
                nc.vector.tensor_copy(out=vT[:, sl], in_=pkv[64:128, :])
                nc.gpsimd.dma_start(out=kT[64:128, sl], in_=kT[0:64, sl])

            def emit_q_proj(i, q0):
                sl = slice(q0, q0 + QCHUNK)
                osl = slice(i * QCHUNK, (i + 1) * QCHUNK)
                pq = ps_kv.tile([64, QCHUNK], f32, tag="kv")
                for c in range(NC_CHUNKS):
                    nc.tensor.matmul(pq, wq[:, c, :], xT[:, c, sl],
                                     start=(c == 0), stop=(c == NC_CHUNKS - 1))
                nc.vector.tensor_copy(out=qT[0:64, osl], in_=pq)
                if _n_kb(q0) >= 12:
                    nc.gpsimd.dma_start(out=qT[64:128, osl], in_=qT[0:64, osl])

            def emit_v_block(kb):
                ptr = ps_tr.tile([128, 64], bf16, tag="tr")
                nc.tensor.transpose(ptr, vT[:, kb * 128:(kb + 1) * 128],
                                    idb[0:64, 0:64])
                nc.vector.tensor_copy(out=vaug[:, kb, 0:H], in_=ptr)

            def emit_attention(i, q0):
                qbase = i * QCHUNK
                nkb = _n_kb(q0)
                d0 = q0 // KB
                o_ps = ps_o.tile([H + 1, QCHUNK], f32)
                for pi in range(nkb // 2):
                    kb1, kb2 = 2 * pi, 2 * pi + 1
                    offs, ns = [], []
                    for kb in (kb1, kb2):
                        d = kb - d0
                        off = 0 if d < 0 else d * KB
                        offs.append(off)
                        ns.append(QCHUNK - off)
                    s_ps = ps_s.tile([128, 2 * QCHUNK], f32)
                    packed = nkb >= 12
                    for half, kb in enumerate((kb1, kb2)):
                        rg = half * 64 if packed else 0
                        nc.tensor.matmul(
                            s_ps[:, half * QCHUNK: half * QCHUNK + ns[half]],
                            kT[rg:rg + 64, kb * KB:(kb + 1) * KB],
                            qT[rg:rg + 64, qbase + offs[half]: qbase + QCHUNK],
                            tile_position=(rg, 0))
                    p_sb = p_pool.tile([128, 2 * QCHUNK], bf16)
                    fd = QCHUNK + ns[1]
                    nc.scalar.activation(out=p_sb[:, 0:fd], in_=s_ps[:, 0:fd],
                                         func=mybir.ActivationFunctionType.Exp,
                                         scale=float(SCALE))
                    for half, kb in enumerate((kb1, kb2)):
                        if kb >= d0:
                            base = half * QCHUNK
                            nc.vector.tensor_mul(p_sb[:, base:base + KB],
                                                 p_sb[:, base:base + KB], mask)
                        nc.tensor.matmul(
                            o_ps[:, offs[half]:QCHUNK],
                            vaug[:, kb, :],
                            p_sb[:, half * QCHUNK: half * QCHUNK + ns[half]],
                            start=(kb == 0), stop=(kb == nkb - 1),
                            skip_group_check=True)

                o_sb = o_pool.tile([H + 1, QCHUNK], f32)
                nc.vector.tensor_copy(out=o_sb, in_=o_ps)
                out_sb = out_pool.tile([128, QCHUNK // 128, H], f32)
                for j in range(QCHUNK // 128):
                    tr = ps_tr.tile([128, H + 1], f32, tag="tr")
                    nc.tensor.transpose(tr, o_sb[:, j * 128:(j + 1) * 128],
                                        idf[0:H + 1, 0:H + 1])
                    rinv = out_pool.tile([128, 1], f32)
                    nc.vector.reciprocal(rinv, tr[:, H:H + 1])
                    nc.vector.tensor_scalar_mul(out_sb[:, j, :], tr[:, 0:H], rinv)
                nc.sync.dma_start(
                    out=y_d[qbase:qbase + QCHUNK, :].rearrange(
                        "(j p) h -> p j h", p=128),
                    in_=out_sb)

            # ---- pipelined emission: each chunk's attention right after
            # the projections/transposes it needs --------------------------
            emitted_t = 0
            emitted_tr = 0
            for ci, q0 in enumerate(q0s):
                need_t = _n_kb(q0) * KB // QCHUNK
                while emitted_t < need_t:
                    emit_kv_proj(emitted_t)
                    emitted_t += 1
                emit_q_proj(ci, q0)
                while emitted_tr < _n_kb(q0):
                    emit_v_block(emitted_tr)
                    emitted_tr += 1
                emit_attention(ci, q0)
    _split_multiwaits(nc)
    return nc


# ---------------------------------------------------------------------------
# PJRT launcher for one Bass program on an arbitrary device subset.
def _make_launcher(nc, devices):
    import jax
    from jax.sharding import Mesh, PartitionSpec
    from jax.experimental.shard_map import shard_map
    import concourse.mybir as mybir
    from concourse.bass2jax import (
        install_neuronx_cc_hook, _bass_exec_p, partition_id_tensor)

    install_neuronx_cc_hook()

    partition_name = nc.partition_id_tensor.name if nc.partition_id_tensor else None
    in_names, out_names, out_avals, zero_outs = [], [], [], []
    for alloc in nc.m.functions[0].allocations:
        if not isinstance(alloc, mybir.MemoryLocationSet):
            continue
        name = alloc.memorylocations[0].name
        if alloc.kind == "ExternalInput":
            if name != partition_name:
                in_names.append(name)
        elif alloc.kind == "ExternalOutput":
            out_names.append(name)
            shape = tuple(alloc.tensor_shape)
            dtype = mybir.dt.np(alloc.dtype)
            out_avals.append(jax.core.ShapedArray(shape, dtype))
            zero_outs.append(np.zeros(shape, dtype))
    n_params = len(in_names)
    n_outs = len(out_avals)
    all_names = in_names + out_names
    if partition_name is not None:
        all_names = all_names + [partition_name]
    donate = tuple(range(n_params, n_params + n_outs))

    def _body(*args):
        operands = list(args)
        if partition_name is not None:
            operands.append(partition_id_tensor())
        outs = _bass_exec_p.bind(
            *operands,
            out_avals=tuple(out_avals),
            in_names=tuple(all_names),
            out_names=tuple(out_names),
            lowering_input_output_aliases=(),
            sim_require_finite=True,
            sim_require_nnan=True,
            nc=nc,
        )
        return tuple(outs)

    n_dev = len(devices)
    mesh = Mesh(np.asarray(devices), ("core",))
    in_specs = (PartitionSpec("core"),) * (n_params + n_outs)
    out_specs = (PartitionSpec("core"),) * n_outs
    fn = jax.jit(
        shard_map(_body, mesh=mesh, in_specs=in_specs, out_specs=out_specs,
                  check_rep=False),
        donate_argnums=donate, keep_unused=True)

    def run(in_maps):
        assert len(in_maps) == n_dev
        concat_in = [
            np.concatenate([np.asarray(in_maps[c][nm]) for c in range(n_dev)], axis=0)
            for nm in in_names
        ]
        concat_zero = [
            np.concatenate([z] * n_dev, axis=0) for z in zero_outs
        ]
        outs = fn(*concat_in, *concat_zero)
        return outs, out_names

    return run


def _get_launchers():
    if "launchers" not in _CACHE:
        import jax
        devs = jax.devices()
        nc0 = _build(0)
        nc1 = _build(1)
        # parity-0 program on devices [0,2,4,6] (batches 0-3),
        # parity-1 on [1,3,5,7].
        run0 = _make_launcher(nc0, [devs[i] for i in (0, 2, 4, 6)])
        run1 = _make_launcher(nc1, [devs[i] for i in (1, 3, 5, 7)])
        _CACHE["launchers"] = (run0, run1)
        _CACHE["ncs"] = (nc0, nc1)
    return _CACHE["launchers"]


def _prep_core_inputs(x, Wq, Wk, Wv):
    x = np.asarray(x, dtype=np.float32)
    wkv = np.concatenate([np.asarray(Wk, np.float32),
                          np.asarray(Wv, np.float32)], axis=1).astype(_BF16)
    wq = np.asarray(Wq, np.float32).astype(_BF16)
    mask = np.triu(np.ones((128, 128), np.float32)).astype(_BF16)
    idb = np.eye(128, dtype=np.float32).astype(_BF16)
    idf = np.eye(128, dtype=np.float32)
    per_batch_xT = [np.ascontiguousarray(x[b].T).astype(_BF16) for b in range(B)]
    common = {"wkv": wkv, "wq": wq, "mask": mask, "idb": idb, "idf": idf}
    maps0 = [{"xT": per_batch_xT[b], **common} for b in range(B)]
    maps1 = [{"xT": per_batch_xT[b], **common} for b in range(B)]
    return maps0, maps1


def kernel(x, Wq, Wk, Wv):
    run0, run1 = _get_launchers()
    maps0, maps1 = _prep_core_inputs(x, Wq, Wk, Wv)
    outs0, names0 = run0(maps0)          # async dispatch
    outs1, names1 = run1(maps1)
    y0 = np.asarray(outs0[names0.index("y")])   # blocks
    y1 = np.asarray(outs1[names1.index("y")])

    out = np.empty((B, T, H), dtype=np.float32)
    rows = 4 * QCHUNK
    for b in range(B):
        yb0 = y0[b * rows:(b + 1) * rows]
        yb1 = y1[b * rows:(b + 1) * rows]
        for i, q0 in enumerate(OWN_Q0[0]):
            out[b, q0:q0 + QCHUNK] = yb0[i * QCHUNK:(i + 1) * QCHUNK]
        for i, q0 in enumerate(OWN_Q0[1]):
            out[b, q0:q0 + QCHUNK] = yb1[i * QCHUNK:(i + 1) * QCHUNK]
    return out
